# revision 26
# baseline (speedup 1.0000x reference)
# Dilated causal self-attention kernel for Trainium2 (8 NeuronCores).
#
# Reference computation (see problem):
#   x (4, 8192, 1024) -> reshape (4, 4, 2048, 1024) -> take every 4th token
#   -> per-segment causal MHA (16 heads, dh=64) -> scatter back into zeros.
#
# Sharding: 16 independent (batch, segment) attention problems, 2 per core.
# Host does the dilated gather + transpose + cast and the final scatter into
# the zero background; each core runs QKV -> per-head causal softmax
# attention -> output projection on its 2 segments.
#
# fp8 usage (e4m3 + DoubleRow): on TRN2 a matmul instruction costs its
# output-column count in cycles regardless of dtype; DoubleRow packs two
# 128-deep contraction chunks into one instruction (2 fp8 weights/cell), so
# an fp8 GEMM needs half the instructions of bf16. fp8's ~2.4%/operand
# quantization noise is only acceptable where softmax averaging damps it:
#   - v GEMM token-chunks 1-3 (keys >= 128): fp8 DR (4 instrs vs 8).
#     Chunk 0 stays bf16: early queries attend to few keys, no averaging.
#   - PV off-diagonal kc1/kc2/kc3 blocks: pT in e4m3 (exp scaled by 2^-4 via
#     bias=-4ln2 so it fits; the ones-column denominator carries the same
#     scale, so normalization cancels it exactly). kc1+kc2 pair into one
#     DoubleRow instr over q in [256,512). kc0 (all early queries + the
#     heavy diagonal) stays bf16.
#   - QKV q/k, scores, and the output projection stay bf16: fp8 noise there
#     lands on the output without averaging and busts the 2e-2 gate.
#
# Device layout (all feature-major where possible):
#   xiT    [C, M]  (per segment)         - input, bf16 (+ e4m3 copy for v)
#   qkT    [2C, M] feature-major         - q rows pre-scaled by 1/sqrt(dh)
#                                          (folded into w_in on host)
#   v      [M, C]  token-major           - v bias folds into output bias
#   scores [128 k, n q] per (head, kc), n = (4-kc)*128 (causal skip)
#   pT = exp(scores - 4ln2); kc0 -> bf16, kc1-3 -> e4m3
#   PV: po[65, M]: rows 0:64 outT, row 64 = denom (ones column in v)
#   yT = w_outT^T @ oT + b_out_eff  -> DMA out feature-major

import sys

sys.path.insert(0, "/opt/trn_rl_repo")

import numpy as np
import ml_dtypes

import concourse.bacc as bacc
import concourse.mybir as mybir
from concourse.tile import TileContext
from concourse.bass_utils import run_bass_kernel_spmd

BF16 = ml_dtypes.bfloat16
E4M3 = ml_dtypes.float8_e4m3

B, N, C = 4, 8192, 1024
W_SEG, RATE, H = 2048, 4, 16
DH = C // H            # 64
S = N // W_SEG         # 4 segments per batch
M = W_SEG // RATE      # 512 tokens per segment
N_CORES = 8
SEG_PER_CORE = (B * S) // N_CORES  # 2

FP32 = mybir.dt.float32
BF = mybir.dt.bfloat16
F8 = mybir.dt.float8e4
DR = mybir.MatmulPerfMode.DoubleRow
EXPB = -4.0 * 0.6931471805599453  # exp scale 2^-4 folded into the bias

_CACHE = {}


def _build():
    nc = bacc.Bacc()

    def mm(out, lhsT, rhs, start, stop, perf_mode=None, skip_group_check=False):
        return nc.tensor.matmul(
            out, lhsT=lhsT, rhs=rhs, start=start, stop=stop,
            perf_mode=perf_mode, skip_group_check=skip_group_check)

    # chunk-major packed layouts (one DMA each; see _prep_inputs)
    xiT = nc.dram_tensor("xiT", [SEG_PER_CORE * 128, 8 * M], BF, kind="ExternalInput")
    xhi = nc.dram_tensor("xhi", [SEG_PER_CORE * 128, 8 * M], F8, kind="ExternalInput")
    wqk = nc.dram_tensor("wqk", [4 * 128, 8 * 512], BF, kind="ExternalInput")
    wv = nc.dram_tensor("wv", [128, 8 * C], BF, kind="ExternalInput")
    wvhi = nc.dram_tensor("wvhi", [128, 8 * C], F8, kind="ExternalInput")
    wout = nc.dram_tensor("wout", [128, 8 * C], BF, kind="ExternalInput")
    bqk = nc.dram_tensor("bqk", [128, 16], FP32, kind="ExternalInput")
    bout = nc.dram_tensor("bout", [128, 8], FP32, kind="ExternalInput")
    yT = nc.dram_tensor("yT", [SEG_PER_CORE * C, M], FP32, kind="ExternalOutput")

    CT = C // 128  # 8 contraction chunks

    from contextlib import ExitStack
    with TileContext(nc) as tc, ExitStack() as ctx:
        consts = ctx.enter_context(tc.tile_pool(name="consts", bufs=1))
        wpool = ctx.enter_context(tc.tile_pool(name="weights", bufs=1))
        xpool = ctx.enter_context(tc.tile_pool(name="x", bufs=1))
        qkpool = ctx.enter_context(tc.tile_pool(name="qk", bufs=24))
        vpool = ctx.enter_context(tc.tile_pool(name="v", bufs=2))
        ptpool = ctx.enter_context(tc.tile_pool(name="pt", bufs=5))
        pt8pool = ctx.enter_context(tc.tile_pool(name="pt8", bufs=4))
        rbpool = ctx.enter_context(tc.tile_pool(name="rb", bufs=3))
        otpool = ctx.enter_context(tc.tile_pool(name="ot", bufs=10))
        ypool = ctx.enter_context(tc.tile_pool(name="y", bufs=3))
        small = ctx.enter_context(tc.tile_pool(name="small", bufs=3))
        psA = ctx.enter_context(tc.tile_pool(name="psA", bufs=2, space="PSUM"))
        psS = ctx.enter_context(tc.tile_pool(name="psS", bufs=3, space="PSUM"))
        psO = ctx.enter_context(tc.tile_pool(name="psO", bufs=3, space="PSUM"))

        bqk_sb = consts.tile([128, 16], FP32, tag="bqk")
        bout_sb = consts.tile([128, 8], FP32, tag="bout")
        expb_sb = consts.tile([128, 1], FP32, tag="expb")
        nc.vector.memset(expb_sb[:], EXPB)

        # wqk_sb[c4][:, ct*512 + off]: weights for qk pair 2*c4+pp,
        # ct-major within the chunk; wv*/wout are [128, ct*1024 + col]
        wqk_sb = [wpool.tile([128, 8 * 512], BF, tag=f"wqk{c4}", name="w")
                  for c4 in range(4)]
        wv_sb = wpool.tile([128, 8 * C], BF, tag="wv")
        wvhi_sb = wpool.tile([128, 8 * C], F8, tag="wvhi")
        wout_sb = wpool.tile([128, 8 * C], BF, tag="wout")

        def emit_w_qk_chunk(c4, eng=None):
            (eng or nc.sync).dma_start(
                out=wqk_sb[c4][:], in_=wqk[c4 * 128:(c4 + 1) * 128, :])

        # --- software-pipelined emission ---------------------------------
        # Dense matmul phases (QKV, proj) are interleaved into the
        # attention phase so the PE never idles:
        #   A(0) | B(0)+C(0) with A(1) spread through | B(1)+C(1)
        x_sb = {}
        xh_sb = {}
        qk_sb = {}
        v8_sb = {}   # [128, 3(kc1..3), 16, 65] e4m3, off-diag PV
        vb_sb = {}   # [128, 16, 65] bf16, kc0 (tokens 0:128)
        oT_sb = {}

        def emit_x(seg):
            t = xpool.tile([128, 8 * M], BF, tag=f"x{seg}", name="x")
            t8 = xpool.tile([128, 8 * M], F8, tag=f"x8{seg}", name="x")
            nc.sync.dma_start(out=t[:],
                              in_=xiT[seg * 128:(seg + 1) * 128, :])
            x_sb[seg] = t
            xh_sb[seg] = t8

        def emit_x8(seg):
            nc.sync.dma_start(out=xh_sb[seg][:],
                              in_=xhi[seg * 128:(seg + 1) * 128, :])

        QK_ORDER = [p + half for p in range(8) for half in (0, 8)]

        def emit_qkv_unit(seg, u):
            # units 0..15: qk e-tiles (interleaved q/k); 16..23: v (tt, nf)
            if u < 16:
                et = QK_ORDER[u]
                p = et % 8
                c4, off = p // 2, (p % 2) * 256 + (0 if et < 8 else 128)
                ps = psA.tile([128, M], FP32, tag="psA", name="ps")
                for ct in range(CT):
                    mm(ps[:],
                       wqk_sb[c4][:, ct * 512 + off:ct * 512 + off + 128],
                       x_sb[seg][:, ct * M:(ct + 1) * M],
                       start=(ct == 0), stop=(ct == CT - 1))
                t = qkpool.tile([128, M], BF, tag="qk", name="qk")
                nc.scalar.activation(
                    out=t[:], in_=ps[:],
                    func=mybir.ActivationFunctionType.Identity,
                    bias=bqk_sb[:, et:et + 1], scale=1.0)
                qk_sb.setdefault(seg, [None] * 16)[et] = t
            else:
                tt, nf = divmod(u - 16, 2)
                if u == 16:
                    v8t = vpool.tile([128, 3, 16, 65], F8, tag="v8", name="v8")
                    vbt = vpool.tile([128, 16, 65], BF, tag="vb", name="vb")
                    v8_sb[seg] = v8t
                    vb_sb[seg] = vbt
                    # ones column per head: PV row 64 accumulates the
                    # softmax denominator for free
                    for kc in range(3):
                        nc.vector.memset(v8t[:, kc, :, 64:65], 1.0)
                    nc.vector.memset(vbt[:, :, 64:65], 1.0)
                v8t, vbt = v8_sb[seg], vb_sb[seg]
                ps = psA.tile([128, M], FP32, tag="psA", name="ps")
                if tt == 0:
                    for ct in range(CT):
                        mm(ps[:],
                           x_sb[seg][:, ct * M:ct * M + 128],
                           wv_sb[:, ct * C + nf * 512:ct * C + (nf + 1) * 512],
                           start=(ct == 0), stop=(ct == CT - 1))
                    nc.scalar.copy(
                        out=vbt[:, nf * 8:(nf + 1) * 8, 0:64],
                        in_=ps[:].rearrange("p (h e) -> p h e", e=64))
                else:
                    xh = xh_sb[seg][:].rearrange("p (g m) -> p g m", m=M)
                    wh = wvhi_sb[:].rearrange("p (g e) -> p g e", e=C)
                    for g in range(4):
                        mm(ps[:],
                           xh[:, 2 * g:2 * g + 2, tt * 128:(tt + 1) * 128],
                           wh[:, 2 * g:2 * g + 2, nf * 512:(nf + 1) * 512],
                           start=(g == 0), stop=(g == 3), perf_mode=DR)
                    # wvhi carries a x16 scale; undo it here
                    nc.vector.tensor_scalar_mul(
                        out=v8t[:, tt - 1, nf * 8:(nf + 1) * 8, 0:64],
                        in0=ps[:].rearrange("p (h e) -> p h e", e=64),
                        scalar1=1.0 / 16.0)

        def emit_scores(seg, h):
            # scoresT blocks [k, q]: lhsT = k-chunk, rhs = q. kc0 -> bf16
            # [128, 512]; kc1-3 -> e4m3 in pt8 [128, 384+256+128]. All carry
            # the 2^-4 exp scale (bias), cancelled by the denominator.
            et, row = h // 2, (h % 2) * 64
            qh = qk_sb[seg][et][row:row + 64, :]
            kh = qk_sb[seg][8 + et][row:row + 64, :]
            pt0 = ptpool.tile([128, M], BF, tag="pt0", name="pt0")
            pt8 = pt8pool.tile([128, 768], F8, tag="pt8", name="pt8")
            OFF8 = [None, 0, 384, 640]
            for kc in range(4):
                n2 = (4 - kc) * 128
                ps = psS.tile([128, M], FP32, tag="psS", name="ps")
                mm(ps[:, :n2], kh[:, kc * 128:(kc + 1) * 128],
                   qh[:, kc * 128:], start=True, stop=True)
                if kc == 0:
                    dst = pt0[:]
                else:
                    dst = pt8[:, OFF8[kc]:OFF8[kc] + n2]
                nc.scalar.activation(
                    out=dst, in_=ps[:, :n2],
                    func=mybir.ActivationFunctionType.Exp, bias=expb_sb[:, 0:1])
                # causal mask: zero the lower triangle of the diagonal
                # block (keep where q_local >= k_local) on idle GpSimd
                nc.gpsimd.affine_select(
                    out=dst[:, 0:128], in_=dst[:, 0:128],
                    compare_op=mybir.AluOpType.is_ge,
                    fill=0.0, base=0,
                    pattern=[[1, 128]], channel_multiplier=-1)
            return pt0, pt8

        def emit_pv(seg, h, po, pt0, pt8):
            # po [65, M]: rows 0:64 = unnormalized outT, row 64 = denom.
            # kc0 (bf16, full width) opens the psum bank; kc1/kc3 are fp8
            # solos, kc1+kc2 pair into one DoubleRow over q in [256,512).
            v8t, vbt = v8_sb[seg], vb_sb[seg]
            mm(po[:], vbt[:, h, :], pt0[:], start=True, stop=False,
               skip_group_check=True)
            mm(po[:, 128:256], v8t[:, 0, h, :], pt8[:, 0:128],
               start=False, stop=False, skip_group_check=True)
            mm(po[:, 256:512], v8t[:, 0:2, h, :],
               pt8[:, 128:640].rearrange("p (i n) -> p i n", n=256),
               start=False, stop=False, perf_mode=DR, skip_group_check=True)
            mm(po[:, 384:512], v8t[:, 2, h, :], pt8[:, 640:768],
               start=False, stop=True, skip_group_check=True)
            denrow = small.tile([1, M], FP32, tag="denrow", name="denrow", bufs=3)
            nc.scalar.copy(out=denrow[:], in_=po[64:65, :])
            rdenT = small.tile([1, M], FP32, tag="rdenT", name="rdenT", bufs=3)
            nc.vector.reciprocal_approx_fast(out=rdenT[:], in_=denrow[:])
            rb = rbpool.tile([64, M], FP32, tag="rb", name="rb")
            nc.gpsimd.partition_broadcast(rb[:], rdenT[:], channels=64)
            row = (h % 2) * 64
            nc.vector.tensor_mul(
                out=oT_sb[seg][h // 2][row:row + 64, :],
                in0=po[0:64, :], in1=rb[:])

        def emit_proj_tile(seg, ot):
            base = seg * C
            ps = psA.tile([128, M], FP32, tag="psA", name="ps")
            for ct in range(CT):
                mm(ps[:],
                   wout_sb[:, ct * C + ot * 128:ct * C + (ot + 1) * 128],
                   oT_sb[seg][ct][:], start=(ct == 0), stop=(ct == CT - 1))
            yt = ypool.tile([128, M], FP32, tag="y", name="yt")
            nc.vector.tensor_scalar_add(yt[:], ps[:], bout_sb[:, ot:ot + 1])
            nc.gpsimd.dma_start(
                out=yT[base + ot * 128:base + (ot + 1) * 128, :], in_=yt[:])

        def emit_attn(seg, filler, warm=None):
            # two-stage software pipeline over heads: scoresT+exp of head
            # h+1 is emitted before PV(h), covering softmax latency.
            # `warm` carries heads whose scores were pre-emitted into the
            # preceding dense stream (pipeline warm-up).
            oT_sb[seg] = [otpool.tile([128, M], BF, tag="ot", name="ot")
                          for _ in range(8)]
            prev = None
            for h in range(H):
                if warm and h in warm:
                    cur = (h,) + warm[h]
                else:
                    pt0, pt8 = emit_scores(seg, h)
                    cur = (h, pt0, pt8,
                           psO.tile([65, M], FP32, tag="psO", name="po"))
                if prev is not None:
                    emit_pv(seg, prev[0], prev[3], prev[1], prev[2])
                filler()
                prev = cur
            emit_pv(seg, prev[0], prev[3], prev[1], prev[2])

        # spread the input DMA issues over idle sequencers so the
        # transfers all start within ~1.2us instead of 600ns apart each
        emit_x(0)
        emit_w_qk_chunk(0)
        nc.sync.dma_start(out=bqk_sb[:], in_=bqk[:, :])
        nc.sync.dma_start(out=bout_sb[:], in_=bout[:, :])
        emit_w_qk_chunk(1)
        nc.sync.dma_start(out=wv_sb[:], in_=wv[:, :])
        emit_w_qk_chunk(2)
        emit_w_qk_chunk(3)
        nc.sync.dma_start(out=wvhi_sb[:], in_=wvhi[:, :])
        emit_x8(0)
        emit_x(1)
        emit_x8(1)
        nc.sync.dma_start(out=wout_sb[:], in_=wout[:, :])
        # seg1 filler interleaves v into the qk stream (weights are long
        # since resident by then); seg0 keeps v last, since the wv DMA
        # lands after the qk weight chunks
        A_ORDER = [0, 1, 16, 2, 3, 17, 4, 5, 18, 6, 7, 19,
                   8, 9, 20, 10, 11, 21, 12, 13, 22, 14, 15, 23]
        for u in range(20):
            emit_qkv_unit(0, u)
        w00 = emit_scores(0, 0)
        warm0 = {0: (w00[0], w00[1],
                     psO.tile([65, M], FP32, tag="psO", name="po"))}
        emit_qkv_unit(0, 20)
        emit_qkv_unit(0, 21)
        w01 = emit_scores(0, 1)
        warm0[1] = (w01[0], w01[1],
                    psO.tile([65, M], FP32, tag="psO", name="po"))
        emit_qkv_unit(0, 22)
        emit_qkv_unit(0, 23)

        # B(0) with A(1) spread through; B(1) with C(0) spread through;
        # C(1) as the dense tail.
        qkv1 = iter(A_ORDER)
        _qcall = [0]

        def fill_qkv1():
            n = 2 if _qcall[0] % 2 == 0 else 1
            _qcall[0] += 1
            for _ in range(n):
                u = next(qkv1, None)
                if u is not None:
                    emit_qkv_unit(1, u)

        emit_attn(0, fill_qkv1, warm=warm0)
        # warm-start seg1's pipeline the same way: its first two heads'
        # score chains begin while seg0's tail PV work runs on the PE
        w10 = emit_scores(1, 0)
        w11 = emit_scores(1, 1)
        warm1 = {0: (w10[0], w10[1],
                     psO.tile([65, M], FP32, tag="psO", name="po")),
                 1: (w11[0], w11[1],
                     psO.tile([65, M], FP32, tag="psO", name="po"))}
        proj0 = iter(range(8))
        _pcall = [0]

        def fill_proj0():
            # emit on odd slots so the filler lasts the whole phase
            if _pcall[0] % 2 == 1:
                ot = next(proj0, None)
                if ot is not None:
                    emit_proj_tile(0, ot)
            _pcall[0] += 1

        emit_attn(1, fill_proj0, warm=warm1)
        for ot in range(8):
            emit_proj_tile(1, ot)

    nc.finalize()
    return nc


def _prep_inputs(x, w_in, b_in, w_out, b_out):
    x = np.asarray(x, dtype=np.float32)
    w_in = np.asarray(w_in, dtype=np.float32)
    b_in = np.asarray(b_in, dtype=np.float32)
    w_out = np.asarray(w_out, dtype=np.float32)
    b_out = np.asarray(b_out, dtype=np.float32)

    # fold 1/sqrt(dh) into the q rows of w_in / b_in
    w_in_s = w_in.copy()
    b_in_s = b_in.copy()
    w_in_s[:C] *= DH ** -0.5
    b_in_s[:C] *= DH ** -0.5

    w_inT0 = np.ascontiguousarray(w_in_s.T).astype(BF16)
    # permute qk columns into [q_p | k_p] pairs matching the consume order
    w_inT = w_inT0.copy()
    for p in range(8):
        w_inT[:, p * 256:p * 256 + 128] = w_inT0[:, p * 128:(p + 1) * 128]
        w_inT[:, p * 256 + 128:(p + 1) * 256] = \
            w_inT0[:, C + p * 128:C + (p + 1) * 128]
    # repack into ct-major chunk layouts (one DMA per chunk on device)
    wp = w_inT.reshape(8, 128, 3 * C)
    wqk = np.ascontiguousarray(np.concatenate(
        [wp[:, :, c4 * 512:(c4 + 1) * 512].transpose(1, 0, 2).reshape(128, 8 * 512)
         for c4 in range(4)], axis=0))                       # (512, 4096)
    wv = np.ascontiguousarray(
        wp[:, :, 2 * C:].transpose(1, 0, 2).reshape(128, 8 * C))  # (128, 8192)
    # e4m3 copy of wv, scaled x16 into fp8-friendly range
    wv16 = np.ascontiguousarray(w_in_s.T)[:, 2 * C:] * 16.0
    wvhi = np.ascontiguousarray(
        np.asarray(wv16, dtype=E4M3).reshape(8, 128, C)
        .transpose(1, 0, 2).reshape(128, 8 * C))
    w_outT = np.ascontiguousarray(w_out.T).astype(BF16)
    wout = np.ascontiguousarray(
        w_outT.reshape(8, 128, C).transpose(1, 0, 2).reshape(128, 8 * C))
    bqk = np.ascontiguousarray(b_in_s[:2 * C].reshape(16, 128).T, dtype=np.float32)
    # v bias folds exactly into an effective output bias:
    #   (p @ (v + 1 b_v^T)) / denom = (p @ v)/denom + b_v
    b_out_eff = b_out + w_out @ b_in[2 * C:]
    bout = np.ascontiguousarray(b_out_eff.reshape(8, 128).T, dtype=np.float32)

    # dilated gather + transpose + ct-major pack: per-core (2*128, 8*M)
    xi = x.reshape(B, S, W_SEG, C)[:, :, ::RATE, :]        # (B, S, M, C)
    xiTf = np.ascontiguousarray(xi.transpose(0, 1, 3, 2))  # (B,S,C,M) fp32

    def pack_x(a):
        return np.ascontiguousarray(
            a.reshape(16, 8, 128, M).transpose(0, 2, 1, 3)
        ).reshape(N_CORES, SEG_PER_CORE * 128, 8 * M)

    xiT = pack_x(xiTf.astype(BF16))
    xhi = pack_x(np.asarray(xiTf, dtype=E4M3))

    in_maps = []
    for c in range(N_CORES):
        in_maps.append({
            "xiT": np.ascontiguousarray(xiT[c]),
            "xhi": np.ascontiguousarray(xhi[c]),
            "wqk": wqk,
            "wv": wv,
            "wvhi": wvhi,
            "wout": wout,
            "bqk": bqk,
            "bout": bout,
        })
    return in_maps


def kernel(x, w_in, b_in, w_out, b_out, _trace=False):
    if "nc" not in _CACHE:
        _CACHE["nc"] = _build()
    nc = _CACHE["nc"]

    in_maps = _prep_inputs(x, w_in, b_in, w_out, b_out)
    res = run_bass_kernel_spmd(
        nc, in_maps, core_ids=list(range(N_CORES)), trace=_trace)
    _CACHE["last_result"] = res

    out = np.zeros((B, N, C), dtype=np.float32)
    ov = out.reshape(B, S, W_SEG, C)
    for c in range(N_CORES):
        yTc = res.results[c]["yT"]                       # (2C, M) fp32
        for seg in range(SEG_PER_CORE):
            gseg = c * SEG_PER_CORE + seg
            b, s = divmod(gseg, S)
            ov[b, s, ::RATE, :] = yTc[seg * C:(seg + 1) * C, :].T
    return out
